# revision 12
# baseline (speedup 1.0000x reference)
"""DeepseekV2 MLA attention (T=2048, H=16) on 8 trn2 cores.

v2: sequence-parallel stage-1 + collectives (vs v1's fully replicated
stage-1, which was ~64% of all PE work).

- Each core runs the low-rank a-projections + RMSNorm + rope for ITS
  T/8=256 tokens only (all heads).
- The normalized kv latent + rope'd k_pe (576 rows x 256 bf16, 327KB)
  is AllGather'd: every core needs all keys for its 2 heads.
- The up-projected, rope'd, normalized q is computed for ALL 16 heads
  on the token-owner core, then AllToAll'd so each core receives its
  2 heads for all T. Per-head q is destination-specific, so AllToAll
  moves 8x less data than gathering the 1536-dim q latent would.
- K/V up-projection, causal attention (2 heads/core) and o_proj are
  unchanged from v1; per-core partial outputs (fp16) are summed on
  the host.

Device-side layout tricks kept from v1: transposed [feature, t]
operands so contractions land on the partition dim; softmax denominator
via ones-matmul; no row-max subtraction; RMS scales applied
post-matmul; neox rope via duplicated/rotated weight columns.
bf16 is used for comm payloads, streamed weights and matmul operands
(same PE speed as fp32r, half the DMA/SBUF/comm bytes).
"""

import numpy as np

T = 2048
HID = 2048
H = 16
NC_ = 8
HLOC = H // NC_          # 2 heads per core
TLOC = T // NC_          # 256 tokens per core
QL = 1536                # q lora
KVL = 512                # kv lora
DN = 128                 # nope dim
DR = 64                  # rope dim
DQK = DN + DR            # 192
DV = 128
EPS = 1e-6
SCALE = float(DQK) ** -0.5
P = 128
QC = 512                 # attention q-chunk
NQC = T // QC
NKB = T // P             # key blocks
NKQ = QL // P            # 12
NKV = KVL // P           # 4
AGR = KVL + DR           # 576 rows in the kv allgather payload
A2R = HLOC * DQK         # 384 rows per a2a block (2 heads x 192)

_CACHE = {}
LAST_RESULTS = None


def _split_multi_waits(nc, mybir):
    """Walrus embeds at most one sem/event wait per TPB instruction; hoist
    extra waits onto preceding same-engine NoOps (queue FIFO keeps order)."""
    n = 0
    for f in nc.m.functions:
        for bb in f.blocks:
            new = []
            for inst in bb.instructions:
                si = getattr(inst, "sync_info", None)
                if si is not None and len(si.on_wait) > 1:
                    waits = list(si.on_wait)
                    for i, wv in enumerate(waits[:-1]):
                        noop = mybir.InstNoOp(
                            name=f"{inst.name}-wsplit{i}",
                            engine=inst.engine,
                            ins=[],
                            outs=[],
                        )
                        noop.bass_nofuse = True
                        noop.sync_info = mybir.SyncInfo(on_wait=[wv], on_update=[])
                        new.append(noop)
                    inst.sync_info = mybir.SyncInfo(
                        on_wait=[waits[-1]], on_update=list(si.on_update)
                    )
                    n += 1
                new.append(inst)
            bb.instructions = new
    return n


def _build_program():
    import concourse.bass as bass
    import concourse.tile as tile
    from concourse import mybir

    f32 = mybir.dt.float32
    bf16 = mybir.dt.bfloat16
    f16 = mybir.dt.float16
    f32r = mybir.dt.float32r
    AF = mybir.ActivationFunctionType

    nc = bass.Bass(num_devices=NC_)

    # local-token inputs / replicated weights (pre-tiled on the host)
    hT_d = nc.declare_dram_parameter("hT", [P, HID // P, TLOC], bf16, isOutput=False)
    wqa_d = nc.declare_dram_parameter("wqa", [P, NKQ, HID // P, P], bf16, isOutput=False)
    # latent 512 | ropeA dup 128 | ropeB dup 128
    wkva_d = nc.declare_dram_parameter("wkva", [P, NKV + 2, HID // P, P], bf16, isOutput=False)
    # all 16 heads: 16 nope tiles then (ropeA_i, ropeB_i) pairs (ln folded)
    wqb_d = nc.declare_dram_parameter("wqb", [P, 32, NKQ, P], bf16, isOutput=False)
    # per-core local heads
    wkvbk_d = nc.declare_dram_parameter("wkvbk", [P, NKV, HLOC * DN], bf16, isOutput=False)
    wkvbv_d = nc.declare_dram_parameter("wkvbv", [P, NKV, HLOC * DV], bf16, isOutput=False)
    wo_d = nc.declare_dram_parameter("wo", [P, HLOC, HID], bf16, isOutput=False)
    cos_d = nc.declare_dram_parameter("cosl", [P, TLOC], f32, isOutput=False)
    sin_d = nc.declare_dram_parameter("sinl", [P, TLOC], f32, isOutput=False)
    trimask_d = nc.declare_dram_parameter("trimask", [P, P], f32, isOutput=False)
    y_d = nc.declare_dram_parameter("y", [T, HID], f16, isOutput=True)

    def r32(ap):
        return ap.bitcast(f32r)

    with tile.TileContext(nc) as tc, nc.allow_low_precision(
        reason="bf16/fp32r matmul operands are intentional"
    ):
        with (
            tc.tile_pool(name="persist", bufs=1) as pp,
            tc.tile_pool(name="dram", bufs=1, space="DRAM") as dp,
        ):
            ag_in = dp.tile([AGR, TLOC], bf16)
            ag_out = dp.tile([NC_ * AGR, TLOC], bf16)
            a2a_in = dp.tile([NC_ * A2R, TLOC], bf16)
            a2a_out = dp.tile([NC_ * A2R, TLOC], bf16)

            # persistent SBUF tensors
            wkvbk_sb = pp.tile([P, NKV, HLOC * DN], bf16, name="wkvbk")
            nc.gpsimd.dma_start(out=wkvbk_sb, in_=wkvbk_d[:, :, :])
            wkvbv_sb = pp.tile([P, NKV, HLOC * DV], bf16, name="wkvbv")
            nc.gpsimd.dma_start(out=wkvbv_sb, in_=wkvbv_d[:, :, :])
            wo_sb = pp.tile([P, HLOC, T], bf16, name="wo")
            nc.gpsimd.dma_start(out=wo_sb, in_=wo_d[:, :, :])
            trimask_sb = pp.tile([P, P], f32, name="trimask")
            nc.gpsimd.dma_start(out=trimask_sb, in_=trimask_d[:, :])
            cos_sb = pp.tile([P, TLOC], f32, name="cosl")
            nc.gpsimd.dma_start(out=cos_sb, in_=cos_d[:, :])
            sin_sb = pp.tile([P, TLOC], f32, name="sinl")
            nc.gpsimd.dma_start(out=sin_sb, in_=sin_d[:, :])
            ones_f = pp.tile([P, P], f32, name="ones_f")
            nc.vector.memset(ones_f, 1.0)
            ones_sb = pp.tile([P, 1], f32r, name="ones")
            nc.vector.tensor_copy(ones_sb, ones_f[:, 0:1])
            col_ones = pp.tile([1, P], f32r, name="col_ones")
            nc.vector.tensor_copy(col_ones, ones_f[0:1, :])
            eps_sb = pp.tile([1, 1], f32, name="eps")
            nc.vector.memset(eps_sb, EPS)

            # gathered / received operands
            qTn = [pp.tile([P, T], bf16, name=f"qTn{h}") for h in range(HLOC)]
            qpeT2 = pp.tile([P, T], bf16, name="qpeT2")  # h0 rows 0:64, h1 64:128
            KT = [pp.tile([P, T], bf16, name=f"KT{h}") for h in range(HLOC)]
            kpe2 = [pp.tile([P, T], bf16, name=f"kpe2{h}") for h in range(HLOC)]
            nc.vector.memset(kpe2[0][DR:P, :], 0.0)
            nc.vector.memset(kpe2[1][0:DR, :], 0.0)
            kva_sb = pp.tile([P, NKV, T], bf16, name="kva")
            V_sb = [pp.tile([P, HLOC * DV], f32r, name=f"v{i}") for i in range(NKB)]

            # ---------------- Stage A: local stage-1 + comm ----------------
            with (
                tc.tile_pool(name="achunk", bufs=1) as ap_,
                tc.tile_pool(name="astream", bufs=3) as sp_,
                tc.tile_pool(name="asmall", bufs=1) as smp,
                tc.tile_pool(name="aps", bufs=2, space="PSUM") as s1ps,
                tc.tile_pool(name="upps", bufs=3, space="PSUM") as upps,
                tc.tile_pool(name="ssqps", bufs=1, space="PSUM") as ssqps,
            ):
                h_sb = ap_.tile([P, HID // P, TLOC], bf16, name="hloc")
                nc.sync.dma_start(out=h_sb[:, 0:8, :], in_=hT_d[:, 0:8, :])
                nc.sync.dma_start(out=h_sb[:, 8:16, :], in_=hT_d[:, 8:16, :])

                ssq_kv = ssqps.tile([1, TLOC], f32, name="ssqkv")
                ssq_q = ssqps.tile([1, TLOC], f32, name="ssqq")

                # --- kv stage-1 first (feeds the AllGather ASAP) ---
                kv_sb = []
                for m in range(NKV + 2):
                    wk_sb = sp_.tile([P, HID // P, P], bf16, name="wstream")
                    nc.sync.dma_start(out=wk_sb, in_=wkva_d[:, m, :, :])
                    ps = s1ps.tile([P, TLOC], f32, name="s1")
                    for k in range(HID // P):
                        nc.tensor.matmul(
                            ps,
                            lhsT=wk_sb[:, k, :],
                            rhs=h_sb[:, k, :],
                            start=(k == 0),
                            stop=(k == HID // P - 1),
                        )
                    lat = smp.tile([P, TLOC], f32, name=f"kvlat{m}")
                    nc.vector.tensor_copy(lat, ps)
                    kv_sb.append(lat)
                    if m < NKV:
                        sq = smp.tile([P, TLOC], f32r, name="sq", bufs=2)
                        nc.scalar.square(sq, ps)
                        nc.tensor.matmul(
                            ssq_kv,
                            lhsT=r32(ones_sb),
                            rhs=r32(sq),
                            start=(m == 0),
                            stop=(m == NKV - 1),
                        )

                # rkv = rsqrt(mean+eps), broadcast over partitions via matmul
                rkv = smp.tile([1, TLOC], f32r, name="rkv")
                nc.scalar.activation(
                    rkv, ssq_kv, func=AF.Sqrt, bias=eps_sb, scale=1.0 / KVL
                )
                nc.vector.reciprocal(rkv, rkv)
                rkvb_ps = upps.tile([P, TLOC], f32, name="up")
                nc.tensor.matmul(rkvb_ps, lhsT=col_ones, rhs=rkv, start=True, stop=True)
                rkv_b = smp.tile([P, TLOC], f32, name="rkvb")
                nc.vector.tensor_copy(rkv_b, rkvb_ps)

                # normalized latent -> bf16 payload tiles; rope'd k_pe
                for m in range(NKV):
                    kvb = smp.tile([P, TLOC], bf16, name=f"kvb{m}")
                    nc.vector.tensor_mul(kvb, kv_sb[m], rkv_b)
                    nc.gpsimd.dma_start(
                        out=ag_in[m * P : (m + 1) * P, :], in_=kvb
                    )
                t1 = smp.tile([P, TLOC], f32, name="ropet1")
                t2 = smp.tile([P, TLOC], f32, name="ropet2")
                nc.vector.tensor_mul(t1, kv_sb[NKV], cos_sb)
                nc.vector.tensor_mul(t2, kv_sb[NKV + 1], sin_sb)
                kpe_bf = smp.tile([P, TLOC], bf16, name="kpebf")
                nc.vector.tensor_add(kpe_bf, t1, t2)
                nc.gpsimd.dma_start(out=ag_in[KVL : KVL + DR, :], in_=kpe_bf[0:DR, :])

                nc.gpsimd.collective_compute(
                    "AllGather",
                    mybir.AluOpType.bypass,
                    replica_groups=[list(range(NC_))],
                    ins=[ag_in[:, :].opt()],
                    outs=[ag_out[:, :].opt()],
                )

                # --- q stage-1 ---
                qc_sb = []
                for m in range(NKQ):
                    wq_sb = sp_.tile([P, HID // P, P], bf16, name="wstream")
                    nc.sync.dma_start(out=wq_sb, in_=wqa_d[:, m, :, :])
                    ps = s1ps.tile([P, TLOC], f32, name="s1")
                    for k in range(HID // P):
                        nc.tensor.matmul(
                            ps,
                            lhsT=wq_sb[:, k, :],
                            rhs=h_sb[:, k, :],
                            start=(k == 0),
                            stop=(k == HID // P - 1),
                        )
                    qt = ap_.tile([P, TLOC], bf16, name=f"qc{m}")
                    nc.vector.tensor_copy(qt, ps)
                    qc_sb.append(qt)
                    sq = smp.tile([P, TLOC], f32r, name="sq", bufs=2)
                    nc.scalar.square(sq, ps)
                    nc.tensor.matmul(
                        ssq_q,
                        lhsT=r32(ones_sb),
                        rhs=r32(sq),
                        start=(m == 0),
                        stop=(m == NKQ - 1),
                    )

                rq = smp.tile([1, TLOC], f32r, name="rq")
                nc.scalar.activation(
                    rq, ssq_q, func=AF.Sqrt, bias=eps_sb, scale=1.0 / QL
                )
                nc.vector.reciprocal(rq, rq)
                rqb_ps = upps.tile([P, TLOC], f32, name="up")
                nc.tensor.matmul(rqb_ps, lhsT=col_ones, rhs=rq, start=True, stop=True)
                rq_b = smp.tile([P, TLOC], f32, name="rqb")
                nc.vector.tensor_copy(rq_b, rqb_ps)

                # --- q up-proj, all 16 heads, local tokens ---
                # mo 0..15: nope head h; mo 16+2i/17+2i: ropeA_i/ropeB_i
                def up_mm(mo):
                    wqbs = sp_.tile([P, NKQ, P], bf16, name="wqbs")
                    nc.sync.dma_start(out=wqbs, in_=wqb_d[:, mo, :, :])
                    ps = upps.tile([P, TLOC], f32, name="up")
                    for k in range(NKQ):
                        nc.tensor.matmul(
                            ps,
                            lhsT=wqbs[:, k, :],
                            rhs=qc_sb[k],
                            start=(k == 0),
                            stop=(k == NKQ - 1),
                        )
                    return ps

                for h in range(H):
                    ps = up_mm(h)
                    qn = smp.tile([P, TLOC], bf16, name=f"qn{h}")
                    nc.vector.tensor_mul(qn, ps, rq_b)
                    e, sl = h // 2, h % 2
                    nc.gpsimd.dma_start(
                        out=a2a_in[e * A2R + sl * DQK : e * A2R + sl * DQK + DN, :],
                        in_=qn,
                    )
                for i in range(H // 2):
                    psA = up_mm(16 + 2 * i)
                    psB = up_mm(17 + 2 * i)
                    t3 = smp.tile([P, TLOC], f32, name="ropet3")
                    t4 = smp.tile([P, TLOC], f32, name="ropet4")
                    nc.vector.tensor_mul(t3, psA, cos_sb)
                    nc.vector.tensor_mul(t4, psB, sin_sb)
                    nc.vector.tensor_add(t3, t3, t4)
                    qr = smp.tile([P, TLOC], bf16, name=f"qr{i}")
                    nc.vector.tensor_mul(qr, t3, rq_b)
                    e = i
                    nc.gpsimd.dma_start(
                        out=a2a_in[e * A2R + DN : e * A2R + DQK, :], in_=qr[0:DR, :]
                    )
                    nc.gpsimd.dma_start(
                        out=a2a_in[e * A2R + DQK + DN : (e + 1) * A2R, :],
                        in_=qr[DR:P, :],
                    )

                nc.gpsimd.collective_compute(
                    "AllToAll",
                    mybir.AluOpType.bypass,
                    replica_groups=[list(range(NC_))],
                    ins=[a2a_in[:, :].opt()],
                    outs=[a2a_out[:, :].opt()],
                )

                # --- unpack AllGather: kva latent + kpe2 ---
                for s in range(NC_):
                    for k in range(NKV):
                        nc.sync.dma_start(
                            out=kva_sb[:, k, s * TLOC : (s + 1) * TLOC],
                            in_=ag_out[s * AGR + k * P : s * AGR + (k + 1) * P, :],
                        )
                    nc.sync.dma_start(
                        out=kpe2[0][0:DR, s * TLOC : (s + 1) * TLOC],
                        in_=ag_out[s * AGR + KVL : s * AGR + KVL + DR, :],
                    )
                    nc.sync.dma_start(
                        out=kpe2[1][DR:P, s * TLOC : (s + 1) * TLOC],
                        in_=ag_out[s * AGR + KVL : s * AGR + KVL + DR, :],
                    )

                # --- kv up-projection: K^T per head, V natural ---
                for h in range(HLOC):
                    for j in range(T // TLOC):
                        ps = upps.tile([P, TLOC], f32, name="up")
                        for k in range(NKV):
                            nc.tensor.matmul(
                                ps,
                                lhsT=wkvbk_sb[:, k, h * P : (h + 1) * P],
                                rhs=kva_sb[:, k, j * TLOC : (j + 1) * TLOC],
                                start=(k == 0),
                                stop=(k == NKV - 1),
                            )
                        nc.vector.tensor_copy(KT[h][:, j * TLOC : (j + 1) * TLOC], ps)
                for tt in range(NKB):
                    ps = upps.tile([P, HLOC * DV], f32, name="up")
                    for k in range(NKV):
                        nc.tensor.matmul(
                            ps,
                            lhsT=kva_sb[:, k, tt * P : (tt + 1) * P],
                            rhs=wkvbv_sb[:, k, :],
                            start=(k == 0),
                            stop=(k == NKV - 1),
                        )
                    nc.vector.tensor_copy(V_sb[tt], ps)

                # --- unpack AllToAll: qTn / qpeT2 for local heads ---
                for s in range(NC_):
                    for h in range(HLOC):
                        nc.sync.dma_start(
                            out=qTn[h][:, s * TLOC : (s + 1) * TLOC],
                            in_=a2a_out[s * A2R + h * DQK : s * A2R + h * DQK + DN, :],
                        )
                        nc.sync.dma_start(
                            out=qpeT2[h * DR : (h + 1) * DR, s * TLOC : (s + 1) * TLOC],
                            in_=a2a_out[
                                s * A2R + h * DQK + DN : s * A2R + (h + 1) * DQK, :
                            ],
                        )

            # ---------------- Stage B: attention ----------------
            with (
                tc.tile_pool(name="bpt", bufs=4) as ptp,
                tc.tile_pool(name="bsmall", bufs=3) as bsm,
                tc.tile_pool(name="sps", bufs=2, space="PSUM") as spsp,
                tc.tile_pool(name="otps", bufs=2, space="PSUM") as otpsp,
                tc.tile_pool(name="lps", bufs=2, space="PSUM") as lpsp,
            ):
                OT_sb = [
                    [ptp.tile([P, QC], bf16, name=f"ot{h}_{j}", bufs=1) for j in range(NQC)]
                    for h in range(HLOC)
                ]

                def flush_norm(pend):
                    p_ot, p_l, p_h, p_j = pend
                    recl = bsm.tile([1, QC], f32r, name="recl")
                    nc.vector.reciprocal(recl, p_l)
                    lb_ps = spsp.tile([P, 2 * QC], f32, name="sps2")[:, :QC]
                    nc.tensor.matmul(lb_ps, lhsT=col_ones, rhs=recl, start=True, stop=True)
                    lb = bsm.tile([P, QC], f32, name="lb")
                    nc.scalar.copy(lb, lb_ps)
                    nc.vector.tensor_mul(OT_sb[p_h][p_j], p_ot, lb)

                pend = None
                for h in range(HLOC):
                    for j in range(NQC):
                        ot_ps = otpsp.tile([P, QC], f32, name="otps")
                        l_ps = lpsp.tile([1, QC], f32, name="lps")
                        nkb = 4 * (j + 1)
                        qcol0 = j * QC
                        for kp in range(0, nkb, 2):
                            # two k-blocks share one PSUM pair and ONE wide exp
                            s2 = spsp.tile([P, 2 * QC], f32, name="sps2")
                            for u in range(2):
                                ki = kp + u
                                nc.tensor.matmul(
                                    s2[:, u * QC : (u + 1) * QC],
                                    lhsT=KT[h][:, ki * P : (ki + 1) * P],
                                    rhs=qTn[h][:, qcol0 : qcol0 + QC],
                                    start=True,
                                    stop=False,
                                )
                                nc.tensor.matmul(
                                    s2[:, u * QC : (u + 1) * QC],
                                    lhsT=kpe2[h][:, ki * P : (ki + 1) * P],
                                    rhs=qpeT2[:, qcol0 : qcol0 + QC],
                                    start=False,
                                    stop=True,
                                )
                            pt = ptp.tile([P, 2 * QC], f32r, name="pt")
                            nc.scalar.activation(pt, s2, func=AF.Exp, scale=SCALE)
                            for u in range(2):
                                ki = kp + u
                                diag = (ki // 4 == j)
                                cs = (ki % 4) * P if diag else 0
                                if diag:
                                    nc.gpsimd.tensor_mul(
                                        pt[:, u * QC + cs : u * QC + cs + P],
                                        pt[:, u * QC + cs : u * QC + cs + P],
                                        trimask_sb,
                                    )
                                nc.tensor.matmul(
                                    ot_ps[:, cs:],
                                    lhsT=V_sb[ki][:, h * DV : (h + 1) * DV],
                                    rhs=pt[:, u * QC + cs : (u + 1) * QC],
                                    start=(ki == 0),
                                    stop=(ki == nkb - 1),
                                )
                                nc.tensor.matmul(
                                    l_ps[:, cs:],
                                    lhsT=r32(ones_sb),
                                    rhs=pt[:, u * QC + cs : (u + 1) * QC],
                                    start=(ki == 0),
                                    stop=(ki == nkb - 1),
                                )
                            if kp == 2 and pend is not None:
                                flush_norm(pend)
                                pend = None
                        pend = (ot_ps, l_ps, h, j)
                flush_norm(pend)

                # ---------------- o_proj ----------------
                for tt in range(T // P):
                    j, sub = tt // 4, (tt % 4) * P
                    for n in range(HID // QC):
                        y_ps = spsp.tile([P, 2 * QC], f32, name="sps2")[:, :QC]
                        for h in range(HLOC):
                            nc.tensor.matmul(
                                y_ps,
                                lhsT=OT_sb[h][j][:, sub : sub + P],
                                rhs=wo_sb[:, h, n * QC : (n + 1) * QC],
                                start=(h == 0),
                                stop=(h == HLOC - 1),
                            )
                        y_sb = ptp.tile([P, QC], f16, name="ysb")
                        nc.scalar.copy(y_sb, y_ps)
                        nc.sync.dma_start(
                            out=y_d[tt * P : (tt + 1) * P, n * QC : (n + 1) * QC],
                            in_=y_sb,
                        )
    _split_multi_waits(nc, mybir)
    return nc


def _host_prep(inputs):
    import ml_dtypes

    hs = np.ascontiguousarray(np.asarray(inputs["hidden_states"], np.float32))
    pos = np.asarray(inputs["positions"], np.int32)
    w_qa = np.asarray(inputs["w_qa"], np.float32)
    q_ln = np.asarray(inputs["q_a_ln_w"], np.float32)
    w_qb = np.asarray(inputs["w_qb"], np.float32)
    w_kva = np.asarray(inputs["w_kva"], np.float32)
    kv_ln = np.asarray(inputs["kv_a_ln_w"], np.float32)
    w_kvb = np.asarray(inputs["w_kvb"], np.float32)
    w_o = np.asarray(inputs["w_o"], np.float32)

    bf = ml_dtypes.bfloat16
    wqa_b = np.ascontiguousarray(
        w_qa.reshape(HID // P, P, QL // P, P).transpose(1, 2, 0, 3)
    ).astype(bf)

    # rope tables (neox), [128, T] with the two 64-halves duplicated
    inv_freq = (1.0 / (10000.0 ** (np.arange(0, DR, 2, dtype=np.float32) / DR))).astype(
        np.float32
    )
    freqs = pos.astype(np.float32)[:, None] * inv_freq[None, :]
    emb = np.concatenate([freqs, freqs], axis=-1)  # [T, 64]
    cosT = np.ascontiguousarray(np.cos(emb).T.astype(np.float32))  # [64, T]
    sinT = np.ascontiguousarray(np.sin(emb).T.astype(np.float32))
    cos2 = np.ascontiguousarray(np.concatenate([cosT, cosT], axis=0))  # [128, T]
    sin2 = np.ascontiguousarray(np.concatenate([sinT, sinT], axis=0))

    def rot_cols(A):
        return np.concatenate([-A[:, DR // 2 :], A[:, : DR // 2]], axis=1)

    # kv a-projection augmented with duplicated rope A/B columns
    kva_lat = w_kva[:, :KVL]
    kva_rope = w_kva[:, KVL:]
    kva_ropeB = rot_cols(kva_rope)
    wkva_aug = np.concatenate(
        [kva_lat, kva_rope, kva_rope, kva_ropeB, kva_ropeB], axis=1
    )
    wkva_b = np.ascontiguousarray(
        wkva_aug.reshape(HID // P, P, NKV + 2, P).transpose(1, 2, 0, 3)
    ).astype(bf)

    w_qb_f = (w_qb * q_ln[:, None]).reshape(QL, H, DQK)
    w_kvb_f = (w_kvb * kv_ln[:, None]).reshape(KVL, H, DN + DV)
    w_o_r = w_o.reshape(H, DV, HID)

    # wqb for ALL heads: 16 nope tiles, then (ropeA_i, ropeB_i) pairs
    cols = [w_qb_f[:, h, :DN] for h in range(H)]
    for i in range(H // 2):
        ra = np.concatenate(
            [w_qb_f[:, 2 * i, DN:], w_qb_f[:, 2 * i + 1, DN:]], axis=1
        )  # [QL, 128]
        rb = np.concatenate(
            [rot_cols(w_qb_f[:, 2 * i, DN:]), rot_cols(w_qb_f[:, 2 * i + 1, DN:])],
            axis=1,
        )
        cols.append(ra)
        cols.append(rb)
    # interleave rope pairs after the 16 nope tiles: order is already
    # nope x16 then A0,B0,A1,B1,...
    wqb_all = np.concatenate(cols, axis=1)  # [QL, 32*128]
    wqb_aug = np.ascontiguousarray(
        wqb_all.reshape(QL // P, P, 32, P).transpose(1, 2, 0, 3)
    ).astype(bf)

    trimask = np.triu(np.ones((P, P), dtype=np.float32))  # [k, q]: 1 iff q>=k

    hTt = hs.reshape(NC_, TLOC, HID // P, P)

    per_core = []
    for i in range(NC_):
        hh = [HLOC * i + x for x in range(HLOC)]
        hT_loc = np.ascontiguousarray(hTt[i].transpose(2, 1, 0)).astype(bf)
        wkvbk = np.ascontiguousarray(
            np.concatenate([w_kvb_f[:, h, :DN] for h in hh], axis=1)
            .reshape(KVL // P, P, HLOC * DN)
            .transpose(1, 0, 2)
        ).astype(bf)
        wkvbv = np.ascontiguousarray(
            np.concatenate([w_kvb_f[:, h, DN:] for h in hh], axis=1)
            .reshape(KVL // P, P, HLOC * DV)
            .transpose(1, 0, 2)
        ).astype(bf)
        wo_i = np.ascontiguousarray(
            np.stack([w_o_r[h] for h in hh], axis=0).transpose(1, 0, 2)
        ).astype(bf)
        per_core.append(
            dict(
                hT=hT_loc,
                wqa=wqa_b,
                wkva=wkva_b,
                wqb=wqb_aug,
                wkvbk=wkvbk,
                wkvbv=wkvbv,
                wo=wo_i,
                cosl=np.ascontiguousarray(cos2[:, i * TLOC : (i + 1) * TLOC]),
                sinl=np.ascontiguousarray(sin2[:, i * TLOC : (i + 1) * TLOC]),
                trimask=trimask,
            )
        )
    return per_core


def kernel(**inputs):
    global LAST_RESULTS
    from concourse.bass_utils import run_bass_kernel_spmd

    if "nc" not in _CACHE:
        _CACHE["nc"] = _build_program()
    nc = _CACHE["nc"]

    in_maps = _host_prep(inputs)
    res = run_bass_kernel_spmd(nc, in_maps, core_ids=list(range(NC_)))
    LAST_RESULTS = res
    out = np.zeros((T, HID), dtype=np.float32)
    for r in res.results:
        out += np.asarray(r["y"], dtype=np.float32)
    return out
